# revision 22
# baseline (speedup 1.0000x reference)
"""Multi-head attention (B=4, N=2048, DIM=768, H=8, DH=96) on 8 TRN2 NeuronCores.

Sharding: data-parallel over (batch, query-half) — core c handles batch c//2,
query rows [(c%2)*1024, (c%2+1)*1024). Each core computes K/V for its full
batch (duplicated across the 2 cores sharing a batch): no collectives.

Q/K projections are UNPADDED (6 dense f-tiles of 128 instead of 8 head-padded
tiles: -36864 PE cycles/core). Heads whose 96 rows live inside one dense
f-tile (h=0,3,4,7; base row 0 or 32, both 32-aligned) read the dense tiles
directly in the dots matmuls; only the 4 straddling heads (1,2,5,6) are
repacked into per-head [96, N] tiles via two full-width SBUF->SBUF DMAs on
the gpsimd queue. Exp runs on [128, 1024] PSUM tiles (2 banks); attnV is
software-pipelined one group behind its exp so the PE never waits on ScalarE.
Weight DMAs are split in halves and interleaved with x quarters so the first
head's K/Q/V never wait on later weights. Head 7 processes qc0's 16 key
tiles first, then qc1's, so its normalize + heads-0..6 projection + head-7
qc0 projection all overlap attention; output is stored bf16 (halves the
output DMA tail) and upcast on the host.
"""

import numpy as np
import ml_dtypes

B, N, DIM = 4, 2048, 768
H, DH = 8, 96
NQ = N // 2
SCALE = DH ** -0.5
NCORES = 8
CT = DIM // 128    # 6 contraction chunks
FT = DIM // 128    # 6 dense f-tiles
NT = N // 128      # 16 key tiles
NQC = NQ // 512    # 2 query chunks

_CACHE = {}


def _head_spans(h):
    """Dense-row pieces covering rows [96h, 96h+96): (f, r0, a, d0)."""
    lo, hi = DH * h, DH * h + DH
    out = []
    for f in range(lo // 128, (hi - 1) // 128 + 1):
        r0 = max(0, lo - 128 * f)
        r1 = min(128, hi - 128 * f)
        out.append((f, r0, r1 - r0, 128 * f + r0 - lo))
    return out


def _f_hi(h):
    return (DH * h + DH - 1) // 128


# heads that can read the dense f-tiles directly in matmuls: single span
# AND base row 0 (ldweights from partition base 32 may touch <=32 partitions)
SINGLE_SPAN = {h for h in range(H)
               if len(_head_spans(h)) == 1 and _head_spans(h)[0][1] == 0}  # {0,4}


def _build():
    import concourse.mybir as mybir
    import concourse.tile as tile
    from concourse import bacc

    f32 = mybir.dt.float32
    bf16 = mybir.dt.bfloat16
    Exp = mybir.ActivationFunctionType.Exp
    mult = mybir.AluOpType.mult
    add = mybir.AluOpType.add

    nc = bacc.Bacc("TRN2", debug=False, num_devices=NCORES)

    xq_d = [nc.dram_tensor(f"xq{i}", [128, CT, 512], bf16, kind="ExternalInput")
            for i in range(4)]
    wka_d = nc.dram_tensor("wka", [128, 3, CT, 128], bf16, kind="ExternalInput")
    wkb_d = nc.dram_tensor("wkb", [128, 3, CT, 128], bf16, kind="ExternalInput")
    wqa_d = nc.dram_tensor("wqa", [128, 3, CT, 128], bf16, kind="ExternalInput")
    wqb_d = nc.dram_tensor("wqb", [128, 3, CT, 128], bf16, kind="ExternalInput")
    wva_d = nc.dram_tensor("wva", [128, CT, 4 * DH], bf16, kind="ExternalInput")
    wvb_d = nc.dram_tensor("wvb", [128, CT, 4 * DH], bf16, kind="ExternalInput")
    wp_d = nc.dram_tensor("wp", [DH + 1, H, DIM], bf16, kind="ExternalInput")
    out_d = nc.dram_tensor("out", [DIM, NQ], bf16, kind="ExternalOutput")

    with tile.TileContext(nc) as tc:
        with (
            tc.tile_pool(name="const", bufs=1) as cpool,
            tc.tile_pool(name="ktdp", bufs=3) as ktd_pool,
            tc.tile_pool(name="qtdp", bufs=3) as qtd_pool,
            tc.tile_pool(name="ktp", bufs=3) as kt_pool,
            tc.tile_pool(name="qtp", bufs=3) as qt_pool,
            tc.tile_pool(name="ptp", bufs=4) as pt_pool,
            tc.tile_pool(name="onp", bufs=16) as on_pool,
            tc.tile_pool(name="smallp", bufs=2) as small_pool,
            tc.tile_pool(name="ysb", bufs=4) as y_pool,
            tc.tile_pool(name="ps_qkv", bufs=2, space="PSUM") as psum_qkv,
            tc.tile_pool(name="ps_d", bufs=2, space="PSUM") as psum_d,
            tc.tile_pool(name="ps_o", bufs=2, space="PSUM") as psum_o,
        ):
            x_sb = [cpool.tile([128, CT, 512], bf16, name=f"x{i}") for i in range(4)]
            wk_sb = cpool.tile([128, FT, CT, 128], bf16, name="wk_sb")
            wq_sb = cpool.tile([128, FT, CT, 128], bf16, name="wq_sb")
            wv_sb = cpool.tile([128, CT, DIM], bf16, name="wv_sb")
            wp_sb = cpool.tile([DH + 1, H, DIM], bf16, name="wp_sb")
            v_sb = [cpool.tile([128, H, DH + 1], bf16, name=f"v{t}") for t in range(NT)]
            y1_sb = [
                [cpool.tile([128, 512], bf16, name=f"y1_{ct}_{qc}") for qc in range(NQC)]
                for ct in range(CT)
            ]

            # input DMAs on the sync queue, ordered by first use
            nc.sync.dma_start(wk_sb[:, 0:3], wka_d.ap())
            nc.sync.dma_start(x_sb[0][:], xq_d[0].ap())
            nc.sync.dma_start(wq_sb[:, 0:3], wqa_d.ap())
            nc.sync.dma_start(x_sb[1][:], xq_d[1].ap())
            nc.sync.dma_start(wv_sb[:, :, 0:4 * DH], wva_d.ap())
            nc.sync.dma_start(x_sb[2][:], xq_d[2].ap())
            nc.sync.dma_start(x_sb[3][:], xq_d[3].ap())
            nc.sync.dma_start(wk_sb[:, 3:6], wkb_d.ap())
            nc.sync.dma_start(wq_sb[:, 3:6], wqb_d.ap())
            nc.sync.dma_start(wv_sb[:, :, 4 * DH:8 * DH], wvb_d.ap())
            nc.sync.dma_start(wp_sb[:], wp_d.ap())

            for t in range(NT):
                nc.vector.memset(v_sb[t][:, :, DH:DH + 1], 1.0)
            ones_bc = cpool.tile([1, DH], bf16, name="ones_bc")
            nc.vector.memset(ones_bc[:], 1.0)

            # PE warmup through the input-DMA window (HAM clock at 8/8)
            warm_sb = cpool.tile([128, 128], bf16, name="warm_sb")
            nc.vector.memset(warm_sb[:], 0.0)
            for _ in range(48):
                wps = psum_qkv.tile([128, 512], f32, name="wps", tag="qkvps")
                nc.tensor.matmul(
                    wps[:, 0:128], lhsT=warm_sb[:], rhs=warm_sb[:],
                    start=True, stop=True,
                )

            ktd = {}   # f -> dense K^T tile [128, N]
            qtd = {}   # f -> dense Q^T tile [128, NQ]
            kt = {}    # h (2-span) -> [96, N]
            qt = {}    # h (2-span) -> [96, NQ]

            def k_ap(h, c0, c1):
                if h in SINGLE_SPAN:
                    (f, r0, a, d0), = _head_spans(h)
                    return ktd[f][r0:r0 + DH, c0:c1]
                return kt[h][:, c0:c1]

            def q_ap(h, c0, c1):
                if h in SINGLE_SPAN:
                    (f, r0, a, d0), = _head_spans(h)
                    return qtd[f][r0:r0 + DH, c0:c1]
                return qt[h][:, c0:c1]

            def k_dense(f, nc_):
                if nc_ == 0:
                    ktd[f] = ktd_pool.tile([128, N], bf16, name="ktd", tag="ktd")
                ps = psum_qkv.tile([128, 512], f32, name="kps", tag="qkvps")
                for ct in range(CT):
                    nc.tensor.matmul(
                        ps,
                        lhsT=wk_sb[:, f, ct, :],
                        rhs=x_sb[nc_][:, ct, :],
                        start=(ct == 0),
                        stop=(ct == CT - 1),
                    )
                nc.scalar.copy(
                    out=ktd[f][:, nc_ * 512:(nc_ + 1) * 512], in_=ps[:]
                )

            def q_dense(f, qc):
                if qc == 0:
                    qtd[f] = qtd_pool.tile([128, NQ], bf16, name="qtd", tag="qtd")
                ps = psum_qkv.tile([128, 512], f32, name="qps", tag="qkvps")
                for ct in range(CT):
                    nc.tensor.matmul(
                        ps,
                        lhsT=wq_sb[:, f, ct, :],
                        rhs=x_sb[qc][:, ct, :],
                        start=(ct == 0),
                        stop=(ct == CT - 1),
                    )
                nc.scalar.copy(
                    out=qtd[f][:, qc * 512:(qc + 1) * 512], in_=ps[:]
                )

            def repack_k(h, half):
                """Repack half the key columns of a straddling head."""
                if h not in kt:
                    kt[h] = kt_pool.tile([DH, N], bf16, name="kt", tag="kt")
                c0, c1 = half * (N // 2), (half + 1) * (N // 2)
                for (f, r0, a, d0) in _head_spans(h):
                    nc.gpsimd.dma_start(
                        kt[h][d0:d0 + a, c0:c1], ktd[f][r0:r0 + a, c0:c1]
                    )

            def repack_q(h):
                qt[h] = qt_pool.tile([DH, NQ], bf16, name="qt", tag="qt")
                for (f, r0, a, d0) in _head_spans(h):
                    nc.gpsimd.dma_start(qt[h][d0:d0 + a, :], qtd[f][r0:r0 + a, :])

            def v_chunk(t, fc):
                ps = psum_qkv.tile([128, 512], f32, name="vps", tag="qkvps")
                vps = ps[:, 0:4 * DH]
                for ct in range(CT):
                    nc.tensor.matmul(
                        vps,
                        lhsT=x_sb[t // 4][:, ct, (t % 4) * 128:(t % 4 + 1) * 128],
                        rhs=wv_sb[:, ct, fc * 4 * DH:(fc + 1) * 4 * DH],
                        start=(ct == 0),
                        stop=(ct == CT - 1),
                    )
                nc.vector.tensor_copy(
                    out=v_sb[t][:, fc * 4:(fc + 1) * 4, 0:DH],
                    in_=ps[:, 0:4 * DH],
                )

            on_sb = {}

            def proj03(ct, qc):
                yp = psum_qkv.tile([128, 512], f32, name="yps", tag="qkvps")
                for h in range(4):
                    nc.tensor.matmul(
                        yp,
                        lhsT=wp_sb[:, h, ct * 128:(ct + 1) * 128],
                        rhs=on_sb[(h, qc)][:],
                        start=(h == 0),
                        stop=(h == 3),
                    )
                nc.vector.tensor_copy(out=y1_sb[ct][qc][:], in_=yp[:])

            def proj46(ct, qc):
                yp = psum_qkv.tile([128, 512], f32, name="yps", tag="qkvps")
                for h in range(4, 7):
                    nc.tensor.matmul(
                        yp,
                        lhsT=wp_sb[:, h, ct * 128:(ct + 1) * 128],
                        rhs=on_sb[(h, qc)][:],
                        start=(h == 4),
                        stop=(h == 6),
                    )
                nc.vector.tensor_tensor(
                    y1_sb[ct][qc][:], y1_sb[ct][qc][:], yp[:], add
                )

            def proj7(ct, qc):
                yp = psum_qkv.tile([128, 512], f32, name="yp7", tag="qkvps")
                nc.tensor.matmul(
                    yp,
                    lhsT=wp_sb[:, 7, ct * 128:(ct + 1) * 128],
                    rhs=on_sb[(7, qc)][:],
                    start=True,
                    stop=True,
                )
                y_sb = y_pool.tile([128, 512], bf16, name="y", tag="y")
                nc.vector.tensor_tensor(y_sb[:], yp[:], y1_sb[ct][qc][:], add)
                nc.sync.dma_start(
                    out_d.ap()[ct * 128:(ct + 1) * 128, qc * 512:(qc + 1) * 512],
                    y_sb[:],
                )

            def evict_o(o_ps_qc):
                """Evict O' and launch the row-sum broadcast at head end.

                s0 rides the gpsimd DMA queue (the sync queue may be deep in
                input/output transfers) and the partition broadcast runs on
                the gpsimd engine — both complete during the next head's
                first groups, so normalize_qc's DVE ops (emitted a few slots
                later) never block the strict-FIFO Vector queue.
                """
                o_st = small_pool.tile(
                    [DH + 1, 512], f32, name="ostage", tag="ostage", bufs=7
                )
                nc.vector.tensor_copy(out=o_st[:], in_=o_ps_qc[:])
                s0 = small_pool.tile([1, 512], f32, name="s0", tag="s0", bufs=4)
                nc.gpsimd.dma_start(s0[:], o_st[DH:DH + 1, :])
                nc.vector.reciprocal_approx_fast(out=s0[:], in_=s0[:])
                s0b = small_pool.tile([1, 512], bf16, name="s0b", tag="s0b", bufs=4)
                nc.vector.tensor_copy(out=s0b[:], in_=s0[:])
                return o_st, s0b

            def normalize_qc(h, qc, o_st, s0b):
                bc = psum_qkv.tile([128, 512], f32, name="bc", tag="qkvps")
                nc.tensor.matmul(
                    bc[0:DH, :], lhsT=ones_bc[:], rhs=s0b[:],
                    start=True, stop=True,
                )
                on = on_pool.tile([DH + 1, 512], bf16, name="on", tag="on")
                on_sb[(h, qc)] = on
                nc.vector.memset(on[DH:DH + 1, :], 1.0)
                nc.vector.tensor_tensor(on[0:DH, :], o_st[0:DH, :], bc[0:DH, :], mult)

            def attn_head(h, fillers):
                """Heads 0-6: 16 groups, one per key tile, both query chunks."""
                o_ps = [
                    psum_o.tile([DH + 1, 512], f32, name=f"ops{qc}", tag="ops")
                    for qc in range(NQC)
                ]
                pending = []
                for t in range(NT):
                    d_ps = psum_d.tile([128, 1024], f32, name="dps", tag="dps")
                    nc.tensor.matmul(
                        d_ps[:, 0:512],
                        lhsT=k_ap(h, t * 128, (t + 1) * 128),
                        rhs=q_ap(h, 0, 512),
                        start=True, stop=True,
                    )
                    nc.tensor.matmul(
                        d_ps[:, 512:1024],
                        lhsT=k_ap(h, t * 128, (t + 1) * 128),
                        rhs=q_ap(h, 512, 1024),
                        start=True, stop=True,
                    )
                    pt = pt_pool.tile([128, 1024], bf16, name="pt", tag="pt")
                    nc.scalar.activation(pt[:], d_ps[:], Exp)
                    for fn in fillers.get(t, ()):
                        fn()
                    for fn in pending:
                        fn()
                    pending = [
                        (lambda tt=t, pp=pt, qc=qc: nc.tensor.matmul(
                            o_ps[qc],
                            lhsT=v_sb[tt][:, h, :],
                            rhs=pp[:, qc * 512:(qc + 1) * 512],
                            start=(tt == 0),
                            stop=(tt == NT - 1),
                        ))
                        for qc in range(NQC)
                    ]
                for fn in pending:
                    fn()

                o_sts = [evict_o(o_ps[qc]) for qc in range(NQC)]

                def finish_normalize(h=h, o_sts=o_sts):
                    for qc in range(NQC):
                        normalize_qc(h, qc, *o_sts[qc])

                return finish_normalize

            def attn_head7(fillers):
                """Head 7: qc0's key tiles (2 per group) first, then qc1's,
                so the qc0 normalize + projection overlap qc1 attention."""
                h = 7
                for qc in range(NQC):
                    o_ps = psum_o.tile([DH + 1, 512], f32, name="ops", tag="ops")
                    pending = []
                    for g in range(NT // 2):
                        slot = qc * (NT // 2) + g
                        t0, t1 = 2 * g, 2 * g + 1
                        d_ps = psum_d.tile([128, 1024], f32, name="dps", tag="dps")
                        nc.tensor.matmul(
                            d_ps[:, 0:512],
                            lhsT=k_ap(h, t0 * 128, (t0 + 1) * 128),
                            rhs=q_ap(h, qc * 512, qc * 512 + 512),
                            start=True, stop=True,
                        )
                        nc.tensor.matmul(
                            d_ps[:, 512:1024],
                            lhsT=k_ap(h, t1 * 128, (t1 + 1) * 128),
                            rhs=q_ap(h, qc * 512, qc * 512 + 512),
                            start=True, stop=True,
                        )
                        pt = pt_pool.tile([128, 1024], bf16, name="pt", tag="pt")
                        nc.scalar.activation(pt[:], d_ps[:], Exp)
                        for fn in fillers.get(slot, ()):
                            fn()
                        for fn in pending:
                            fn()
                        pending = [
                            (lambda tt=tt, j=j, pp=pt, oo=o_ps: nc.tensor.matmul(
                                oo,
                                lhsT=v_sb[tt][:, h, :],
                                rhs=pp[:, j * 512:(j + 1) * 512],
                                start=(tt == 0),
                                stop=(tt == NT - 1),
                            ))
                            for j, tt in ((0, t0), (1, t1))
                        ]
                    for fn in pending:
                        fn()
                    o_st, s0 = evict_o(o_ps)
                    normalize_qc(h, qc, o_st, s0)
                    if qc == 0:
                        # overlap head-7 qc0 projection with qc1 attention;
                        # start after the qc0 normalize chain (~3us) lands
                        for i, c in enumerate(range(CT)):
                            fillers.setdefault(10 + min(i // 2, 2), []).append(
                                lambda c=c: proj46(c, 0)
                            )
                        for i, c in enumerate(range(4)):
                            fillers.setdefault(12 + i, []).append(
                                lambda c=c: proj7(c, 0)
                            )

            # ---- preamble: head-0 critical path (h0 reads dense tiles) ----
            k_dense(0, 0)
            q_dense(0, 0)
            q_dense(0, 1)
            v_chunk(0, 0)
            v_chunk(1, 0)

            def mk_fillers(h):
                f = {}

                def addf(slot, fn):
                    f.setdefault(slot, []).append(fn)

                if h == 0:
                    for n in (1, 2, 3):
                        addf(4 * n - 4, lambda n=n: k_dense(0, n))
                    for t in range(2, NT):
                        addf(max(0, t - 2), lambda tt=t: v_chunk(tt, 0))
                    for i, n in enumerate((0, 1, 2, 3)):
                        addf(2 * i + 3, lambda n=n: k_dense(1, n))
                    addf(6, lambda: repack_k(1, 0))
                    addf(10, lambda: repack_k(1, 1))
                    addf(10, lambda: q_dense(1, 0))
                    addf(11, lambda: q_dense(1, 1))
                    addf(12, lambda: repack_q(1))
                elif h in (1, 2, 3, 5):
                    fd = h + 1 if h != 5 else 5
                    if h == 2:
                        # h3 is a single straddle-free span at base row 32 of
                        # f2 (done during head 1) but still needs the
                        # partition shift to base 0
                        addf(1, lambda: repack_k(3, 0))
                        addf(2, lambda: repack_k(3, 1))
                        addf(3, lambda: repack_q(3))
                    for i, n in enumerate((0, 1, 2, 3)):
                        addf(2 * i + 1, lambda n=n, fd=fd: k_dense(fd, n))
                    addf(9, lambda fd=fd: q_dense(fd, 0))
                    addf(10, lambda fd=fd: q_dense(fd, 1))
                    # repack 2-span heads once their f_hi dense tiles exist
                    for hh in [hh for hh in range(H) if hh not in SINGLE_SPAN
                               and _f_hi(hh) == fd]:
                        addf(5, lambda hh=hh: repack_k(hh, 0))
                        addf(9, lambda hh=hh: repack_k(hh, 1))
                        addf(11, lambda hh=hh: repack_q(hh))
                if h in (1, 2, 3, 4):
                    for i in range(4):
                        addf(2 * i + 2, lambda tt=4 * (h - 1) + i: v_chunk(tt, 1))
                if h == 4:
                    for i, c in enumerate(range(CT)):
                        addf(2 * i + 5, lambda c=c: proj03(c, 0))
                if h == 6:
                    for i, c in enumerate(range(CT)):
                        addf(2 * i + 1, lambda c=c: proj03(c, 1))
                if h == 7:
                    # heads 4-6 qc1 projection during head-7 qc0 attention
                    # (8 groups); qc0 proj is appended inside attn_head7.
                    for i, c in enumerate(range(CT)):
                        addf(min(2 + i, 7), lambda c=c: proj46(c, 1))
                return f

            norm_prev = None
            for h in range(H - 1):
                f = mk_fillers(h)
                if norm_prev is not None:
                    f.setdefault(4, []).insert(0, norm_prev)
                norm_prev = attn_head(h, f)
            f7 = mk_fillers(7)
            f7.setdefault(1, []).insert(0, norm_prev)
            attn_head7(f7)

            # ---- tail: head-7 qc1 projection + out ----
            proj7(4, 0)
            proj7(5, 0)
            for ct in range(CT):
                proj7(ct, 1)

    nc.compile()
    return nc


def _get_nc():
    if "nc" not in _CACHE:
        _CACHE["nc"] = _build()
    return _CACHE["nc"]


def _prep_shards(x, w_qkv, w_proj, b_proj):
    bf16 = ml_dtypes.bfloat16
    x = np.asarray(x, dtype=np.float32)
    w_qkv = np.asarray(w_qkv, dtype=np.float32)
    w_proj = np.asarray(w_proj, dtype=np.float32)
    b_proj = np.asarray(b_proj, dtype=np.float32)

    def fmajor(w):  # [768c, 768f] -> [128, FT, CT, 128]: (p,f,ct,j) = w[ct*128+p, f*128+j]
        a = w.reshape(CT, 128, FT, 128)
        return np.ascontiguousarray(a.transpose(1, 2, 0, 3)).astype(bf16)

    def pmajor(w):  # [768c, F] -> [128, CT, F]
        return np.ascontiguousarray(
            w.reshape(CT, 128, w.shape[1]).transpose(1, 0, 2)
        ).astype(bf16)

    wq_b = fmajor(w_qkv[0:DIM].T * SCALE)
    wk_b = fmajor(w_qkv[DIM:2 * DIM].T)
    wv_b = pmajor(w_qkv[2 * DIM:3 * DIM].T)
    wp_arr = np.zeros((DH + 1, H, DIM), np.float32)
    wp_arr[0:DH] = w_proj.T.reshape(H, DH, DIM).transpose(1, 0, 2)
    wp_arr[DH, 0, :] = b_proj
    wp_b = np.ascontiguousarray(wp_arr).astype(bf16)

    in_maps = []
    for c in range(NCORES):
        b, half = divmod(c, 2)
        xt = x[b].T  # [768, 2048]
        if half == 1:
            xt = np.concatenate([xt[:, NQ:], xt[:, :NQ]], axis=1)
        xq = pmajor(xt)  # [128, CT, 2048]
        im = {
            "wka": np.ascontiguousarray(wk_b[:, 0:3]),
            "wkb": np.ascontiguousarray(wk_b[:, 3:6]),
            "wqa": np.ascontiguousarray(wq_b[:, 0:3]),
            "wqb": np.ascontiguousarray(wq_b[:, 3:6]),
            "wva": np.ascontiguousarray(wv_b[:, :, 0:4 * DH]),
            "wvb": np.ascontiguousarray(wv_b[:, :, 4 * DH:8 * DH]),
            "wp": wp_b,
        }
        for i in range(4):
            im[f"xq{i}"] = np.ascontiguousarray(xq[:, :, i * 512:(i + 1) * 512])
        in_maps.append(im)
    return in_maps


def kernel(x, w_qkv, w_proj, b_proj):
    from concourse.bass_utils import run_bass_kernel_spmd

    nc = _get_nc()
    in_maps = _prep_shards(x, w_qkv, w_proj, b_proj)
    res = run_bass_kernel_spmd(nc, in_maps, core_ids=list(range(NCORES)))
    out = np.empty((B, N, DIM), np.float32)
    for c in range(NCORES):
        b, half = divmod(c, 2)
        yT = np.asarray(res.results[c]["out"], dtype=np.float32)  # [768, 1024]
        out[b, half * NQ:(half + 1) * NQ, :] = yT.T
    return out


# revision 23
# speedup vs baseline: 1.2079x; 1.2079x over previous
"""Multi-head attention (B=4, N=2048, DIM=768, H=8, DH=96) on 8 TRN2 NeuronCores.

Sharding: data-parallel over (batch, query-half) — core c handles batch c//2,
query rows [(c%2)*1024, (c%2+1)*1024). Each core computes K/V for its full
batch (duplicated across the 2 cores sharing a batch): no collectives.

Q/K projections are UNPADDED (6 dense f-tiles of 128 instead of 8 head-padded
tiles: -36864 PE cycles/core). Heads whose 96 rows live inside one dense
f-tile (h=0,3,4,7; base row 0 or 32, both 32-aligned) read the dense tiles
directly in the dots matmuls; only the 4 straddling heads (1,2,5,6) are
repacked into per-head [96, N] tiles via two full-width SBUF->SBUF DMAs on
the gpsimd queue. Exp runs on [128, 1024] PSUM tiles (2 banks); attnV is
software-pipelined one group behind its exp so the PE never waits on ScalarE.
Weight DMAs are split in halves and interleaved with x quarters so the first
head's K/Q/V never wait on later weights. Head 7 processes qc0's 16 key
tiles first, then qc1's, so its normalize + heads-0..6 projection + head-7
qc0 projection all overlap attention; output is stored bf16 (halves the
output DMA tail) and upcast on the host.
"""

import numpy as np
import ml_dtypes

B, N, DIM = 4, 2048, 768
H, DH = 8, 96
NQ = N // 2
SCALE = DH ** -0.5
NCORES = 8
CT = DIM // 128    # 6 contraction chunks
FT = DIM // 128    # 6 dense f-tiles
NT = N // 128      # 16 key tiles
NQC = NQ // 512    # 2 query chunks

_CACHE = {}


def _head_spans(h):
    """Dense-row pieces covering rows [96h, 96h+96): (f, r0, a, d0)."""
    lo, hi = DH * h, DH * h + DH
    out = []
    for f in range(lo // 128, (hi - 1) // 128 + 1):
        r0 = max(0, lo - 128 * f)
        r1 = min(128, hi - 128 * f)
        out.append((f, r0, r1 - r0, 128 * f + r0 - lo))
    return out


def _f_hi(h):
    return (DH * h + DH - 1) // 128


# heads that can read the dense f-tiles directly in matmuls: single span
# AND base row 0 (ldweights from partition base 32 may touch <=32 partitions)
SINGLE_SPAN = {h for h in range(H)
               if len(_head_spans(h)) == 1 and _head_spans(h)[0][1] == 0}  # {0,4}


def _build():
    import concourse.mybir as mybir
    import concourse.tile as tile
    from concourse import bacc

    f32 = mybir.dt.float32
    bf16 = mybir.dt.bfloat16
    Exp = mybir.ActivationFunctionType.Exp
    mult = mybir.AluOpType.mult
    add = mybir.AluOpType.add

    nc = bacc.Bacc("TRN2", debug=False, num_devices=NCORES)

    xq_d = [nc.dram_tensor(f"xq{i}", [128, CT, 512], bf16, kind="ExternalInput")
            for i in range(4)]
    wka_d = nc.dram_tensor("wka", [128, 3, CT, 128], bf16, kind="ExternalInput")
    wkb_d = nc.dram_tensor("wkb", [128, 3, CT, 128], bf16, kind="ExternalInput")
    wqa_d = nc.dram_tensor("wqa", [128, 3, CT, 128], bf16, kind="ExternalInput")
    wqb_d = nc.dram_tensor("wqb", [128, 3, CT, 128], bf16, kind="ExternalInput")
    wva_d = nc.dram_tensor("wva", [128, CT, 4 * DH], bf16, kind="ExternalInput")
    wvb_d = nc.dram_tensor("wvb", [128, CT, 4 * DH], bf16, kind="ExternalInput")
    wp_d = nc.dram_tensor("wp", [DH + 1, H, DIM], bf16, kind="ExternalInput")
    out_d = nc.dram_tensor("out", [DIM, NQ], bf16, kind="ExternalOutput")

    with tile.TileContext(nc) as tc:
        with (
            tc.tile_pool(name="const", bufs=1) as cpool,
            tc.tile_pool(name="ktdp", bufs=3) as ktd_pool,
            tc.tile_pool(name="qtdp", bufs=3) as qtd_pool,
            tc.tile_pool(name="ktp", bufs=3) as kt_pool,
            tc.tile_pool(name="qtp", bufs=3) as qt_pool,
            tc.tile_pool(name="ptp", bufs=4) as pt_pool,
            tc.tile_pool(name="onp", bufs=16) as on_pool,
            tc.tile_pool(name="smallp", bufs=2) as small_pool,
            tc.tile_pool(name="ysb", bufs=4) as y_pool,
            tc.tile_pool(name="ps_qkv", bufs=2, space="PSUM") as psum_qkv,
            tc.tile_pool(name="ps_d", bufs=2, space="PSUM") as psum_d,
            tc.tile_pool(name="ps_o", bufs=2, space="PSUM") as psum_o,
        ):
            x_sb = [cpool.tile([128, CT, 512], bf16, name=f"x{i}") for i in range(4)]
            wk_sb = cpool.tile([128, FT, CT, 128], bf16, name="wk_sb")
            wq_sb = cpool.tile([128, FT, CT, 128], bf16, name="wq_sb")
            wv_sb = cpool.tile([128, CT, DIM], bf16, name="wv_sb")
            wp_sb = cpool.tile([DH + 1, H, DIM], bf16, name="wp_sb")
            v_sb = [cpool.tile([128, H, DH + 1], bf16, name=f"v{t}") for t in range(NT)]
            y1_sb = [
                [cpool.tile([128, 512], bf16, name=f"y1_{ct}_{qc}") for qc in range(NQC)]
                for ct in range(CT)
            ]

            # input DMAs on the sync queue, ordered by first use
            nc.sync.dma_start(wk_sb[:, 0:3], wka_d.ap())
            nc.sync.dma_start(x_sb[0][:], xq_d[0].ap())
            nc.sync.dma_start(wq_sb[:, 0:3], wqa_d.ap())
            nc.sync.dma_start(x_sb[1][:], xq_d[1].ap())
            nc.sync.dma_start(wv_sb[:, :, 0:4 * DH], wva_d.ap())
            nc.sync.dma_start(x_sb[2][:], xq_d[2].ap())
            nc.sync.dma_start(x_sb[3][:], xq_d[3].ap())
            nc.sync.dma_start(wk_sb[:, 3:6], wkb_d.ap())
            nc.sync.dma_start(wq_sb[:, 3:6], wqb_d.ap())
            nc.sync.dma_start(wv_sb[:, :, 4 * DH:8 * DH], wvb_d.ap())
            nc.sync.dma_start(wp_sb[:], wp_d.ap())

            for t in range(NT):
                nc.vector.memset(v_sb[t][:, :, DH:DH + 1], 1.0)
            ones_bc = cpool.tile([1, DH], bf16, name="ones_bc")
            nc.vector.memset(ones_bc[:], 1.0)

            # PE warmup through the input-DMA window (HAM clock at 8/8)
            warm_sb = cpool.tile([128, 128], bf16, name="warm_sb")
            nc.vector.memset(warm_sb[:], 0.0)
            for _ in range(48):
                wps = psum_qkv.tile([128, 512], f32, name="wps", tag="qkvps")
                nc.tensor.matmul(
                    wps[:, 0:128], lhsT=warm_sb[:], rhs=warm_sb[:],
                    start=True, stop=True,
                )

            ktd = {}   # f -> dense K^T tile [128, N]
            qtd = {}   # f -> dense Q^T tile [128, NQ]
            kt = {}    # h (2-span) -> [96, N]
            qt = {}    # h (2-span) -> [96, NQ]

            def k_ap(h, c0, c1):
                if h in SINGLE_SPAN:
                    (f, r0, a, d0), = _head_spans(h)
                    return ktd[f][r0:r0 + DH, c0:c1]
                return kt[h][:, c0:c1]

            def q_ap(h, c0, c1):
                if h in SINGLE_SPAN:
                    (f, r0, a, d0), = _head_spans(h)
                    return qtd[f][r0:r0 + DH, c0:c1]
                return qt[h][:, c0:c1]

            def k_dense(f, nc_):
                if nc_ == 0:
                    ktd[f] = ktd_pool.tile([128, N], bf16, name="ktd", tag="ktd")
                ps = psum_qkv.tile([128, 512], f32, name="kps", tag="qkvps")
                for ct in range(CT):
                    nc.tensor.matmul(
                        ps,
                        lhsT=wk_sb[:, f, ct, :],
                        rhs=x_sb[nc_][:, ct, :],
                        start=(ct == 0),
                        stop=(ct == CT - 1),
                    )
                nc.scalar.copy(
                    out=ktd[f][:, nc_ * 512:(nc_ + 1) * 512], in_=ps[:]
                )

            def q_dense(f, qc):
                if qc == 0:
                    qtd[f] = qtd_pool.tile([128, NQ], bf16, name="qtd", tag="qtd")
                ps = psum_qkv.tile([128, 512], f32, name="qps", tag="qkvps")
                for ct in range(CT):
                    nc.tensor.matmul(
                        ps,
                        lhsT=wq_sb[:, f, ct, :],
                        rhs=x_sb[qc][:, ct, :],
                        start=(ct == 0),
                        stop=(ct == CT - 1),
                    )
                nc.scalar.copy(
                    out=qtd[f][:, qc * 512:(qc + 1) * 512], in_=ps[:]
                )

            def repack_k(h, half):
                """Repack half the key columns of a straddling head."""
                if h not in kt:
                    kt[h] = kt_pool.tile([DH, N], bf16, name="kt", tag="kt")
                c0, c1 = half * (N // 2), (half + 1) * (N // 2)
                for (f, r0, a, d0) in _head_spans(h):
                    nc.gpsimd.dma_start(
                        kt[h][d0:d0 + a, c0:c1], ktd[f][r0:r0 + a, c0:c1]
                    )

            def repack_q(h):
                qt[h] = qt_pool.tile([DH, NQ], bf16, name="qt", tag="qt")
                for (f, r0, a, d0) in _head_spans(h):
                    nc.gpsimd.dma_start(qt[h][d0:d0 + a, :], qtd[f][r0:r0 + a, :])

            def v_chunk(t, fc):
                ps = psum_qkv.tile([128, 512], f32, name="vps", tag="qkvps")
                vps = ps[:, 0:4 * DH]
                for ct in range(CT):
                    nc.tensor.matmul(
                        vps,
                        lhsT=x_sb[t // 4][:, ct, (t % 4) * 128:(t % 4 + 1) * 128],
                        rhs=wv_sb[:, ct, fc * 4 * DH:(fc + 1) * 4 * DH],
                        start=(ct == 0),
                        stop=(ct == CT - 1),
                    )
                nc.vector.tensor_copy(
                    out=v_sb[t][:, fc * 4:(fc + 1) * 4, 0:DH],
                    in_=ps[:, 0:4 * DH],
                )

            on_sb = {}

            def proj03(ct, qc):
                yp = psum_qkv.tile([128, 512], f32, name="yps", tag="qkvps")
                for h in range(4):
                    nc.tensor.matmul(
                        yp,
                        lhsT=wp_sb[:, h, ct * 128:(ct + 1) * 128],
                        rhs=on_sb[(h, qc)][:],
                        start=(h == 0),
                        stop=(h == 3),
                    )
                nc.vector.tensor_copy(out=y1_sb[ct][qc][:], in_=yp[:])

            def proj46(ct, qc):
                yp = psum_qkv.tile([128, 512], f32, name="yps", tag="qkvps")
                for h in range(4, 7):
                    nc.tensor.matmul(
                        yp,
                        lhsT=wp_sb[:, h, ct * 128:(ct + 1) * 128],
                        rhs=on_sb[(h, qc)][:],
                        start=(h == 4),
                        stop=(h == 6),
                    )
                nc.vector.tensor_tensor(
                    y1_sb[ct][qc][:], y1_sb[ct][qc][:], yp[:], add
                )

            def proj7(ct, qc):
                yp = psum_qkv.tile([128, 512], f32, name="yp7", tag="qkvps")
                nc.tensor.matmul(
                    yp,
                    lhsT=wp_sb[:, 7, ct * 128:(ct + 1) * 128],
                    rhs=on_sb[(7, qc)][:],
                    start=True,
                    stop=True,
                )
                y_sb = y_pool.tile([128, 512], bf16, name="y", tag="y")
                nc.vector.tensor_tensor(y_sb[:], yp[:], y1_sb[ct][qc][:], add)
                nc.sync.dma_start(
                    out_d.ap()[ct * 128:(ct + 1) * 128, qc * 512:(qc + 1) * 512],
                    y_sb[:],
                )

            def evict_o(o_ps_qc):
                """Evict O' and launch the row-sum broadcast at head end.

                s0 rides the gpsimd DMA queue (the sync queue may be deep in
                input/output transfers) and the partition broadcast runs on
                the gpsimd engine — both complete during the next head's
                first groups, so normalize_qc's DVE ops (emitted a few slots
                later) never block the strict-FIFO Vector queue.
                """
                o_st = small_pool.tile(
                    [DH + 1, 512], f32, name="ostage", tag="ostage", bufs=7
                )
                nc.vector.tensor_copy(out=o_st[:], in_=o_ps_qc[:])
                s0 = small_pool.tile([1, 512], f32, name="s0", tag="s0", bufs=4)
                nc.gpsimd.dma_start(s0[:], o_st[DH:DH + 1, :])
                sb = small_pool.tile([DH, 512], f32, name="sbc", tag="sbc", bufs=4)
                nc.gpsimd.partition_broadcast(sb[:], s0[:])
                return o_st, sb

            def normalize_qc(h, qc, o_st, sb):
                nc.vector.reciprocal_approx_fast(out=sb[:], in_=sb[:])
                on = on_pool.tile([DH + 1, 512], bf16, name="on", tag="on")
                on_sb[(h, qc)] = on
                nc.vector.memset(on[DH:DH + 1, :], 1.0)
                nc.vector.tensor_tensor(on[0:DH, :], o_st[0:DH, :], sb[:], mult)

            def attn_head(h, fillers):
                """Heads 0-6: 16 groups, one per key tile, both query chunks."""
                o_ps = [
                    psum_o.tile([DH + 1, 512], f32, name=f"ops{qc}", tag="ops")
                    for qc in range(NQC)
                ]
                pending = []
                for t in range(NT):
                    d_ps = psum_d.tile([128, 1024], f32, name="dps", tag="dps")
                    nc.tensor.matmul(
                        d_ps[:, 0:512],
                        lhsT=k_ap(h, t * 128, (t + 1) * 128),
                        rhs=q_ap(h, 0, 512),
                        start=True, stop=True,
                    )
                    nc.tensor.matmul(
                        d_ps[:, 512:1024],
                        lhsT=k_ap(h, t * 128, (t + 1) * 128),
                        rhs=q_ap(h, 512, 1024),
                        start=True, stop=True,
                    )
                    pt = pt_pool.tile([128, 1024], bf16, name="pt", tag="pt")
                    nc.scalar.activation(pt[:], d_ps[:], Exp)
                    for fn in fillers.get(t, ()):
                        fn()
                    for fn in pending:
                        fn()
                    pending = [
                        (lambda tt=t, pp=pt, qc=qc: nc.tensor.matmul(
                            o_ps[qc],
                            lhsT=v_sb[tt][:, h, :],
                            rhs=pp[:, qc * 512:(qc + 1) * 512],
                            start=(tt == 0),
                            stop=(tt == NT - 1),
                        ))
                        for qc in range(NQC)
                    ]
                for fn in pending:
                    fn()

                o_sts = [evict_o(o_ps[qc]) for qc in range(NQC)]

                def finish_normalize(h=h, o_sts=o_sts):
                    for qc in range(NQC):
                        normalize_qc(h, qc, *o_sts[qc])

                return finish_normalize

            def attn_head7(fillers):
                """Head 7: qc0's key tiles (2 per group) first, then qc1's,
                so the qc0 normalize + projection overlap qc1 attention."""
                h = 7
                for qc in range(NQC):
                    o_ps = psum_o.tile([DH + 1, 512], f32, name="ops", tag="ops")
                    pending = []
                    for g in range(NT // 2):
                        slot = qc * (NT // 2) + g
                        t0, t1 = 2 * g, 2 * g + 1
                        d_ps = psum_d.tile([128, 1024], f32, name="dps", tag="dps")
                        nc.tensor.matmul(
                            d_ps[:, 0:512],
                            lhsT=k_ap(h, t0 * 128, (t0 + 1) * 128),
                            rhs=q_ap(h, qc * 512, qc * 512 + 512),
                            start=True, stop=True,
                        )
                        nc.tensor.matmul(
                            d_ps[:, 512:1024],
                            lhsT=k_ap(h, t1 * 128, (t1 + 1) * 128),
                            rhs=q_ap(h, qc * 512, qc * 512 + 512),
                            start=True, stop=True,
                        )
                        pt = pt_pool.tile([128, 1024], bf16, name="pt", tag="pt")
                        nc.scalar.activation(pt[:], d_ps[:], Exp)
                        for fn in fillers.get(slot, ()):
                            fn()
                        for fn in pending:
                            fn()
                        pending = [
                            (lambda tt=tt, j=j, pp=pt, oo=o_ps: nc.tensor.matmul(
                                oo,
                                lhsT=v_sb[tt][:, h, :],
                                rhs=pp[:, j * 512:(j + 1) * 512],
                                start=(tt == 0),
                                stop=(tt == NT - 1),
                            ))
                            for j, tt in ((0, t0), (1, t1))
                        ]
                    for fn in pending:
                        fn()
                    o_st, s0 = evict_o(o_ps)
                    normalize_qc(h, qc, o_st, s0)
                    if qc == 0:
                        # overlap head-7 qc0 projection with qc1 attention;
                        # start after the qc0 normalize chain (~3us) lands
                        for i, c in enumerate(range(CT)):
                            fillers.setdefault(10 + min(i // 2, 2), []).append(
                                lambda c=c: proj46(c, 0)
                            )
                        for i, c in enumerate(range(4)):
                            fillers.setdefault(12 + i, []).append(
                                lambda c=c: proj7(c, 0)
                            )

            # ---- preamble: head-0 critical path (h0 reads dense tiles) ----
            k_dense(0, 0)
            q_dense(0, 0)
            q_dense(0, 1)
            v_chunk(0, 0)
            v_chunk(1, 0)

            def mk_fillers(h):
                f = {}

                def addf(slot, fn):
                    f.setdefault(slot, []).append(fn)

                if h == 0:
                    for n in (1, 2, 3):
                        addf(4 * n - 4, lambda n=n: k_dense(0, n))
                    for t in range(2, NT):
                        addf(max(0, t - 2), lambda tt=t: v_chunk(tt, 0))
                    for i, n in enumerate((0, 1, 2, 3)):
                        addf(2 * i + 3, lambda n=n: k_dense(1, n))
                    addf(6, lambda: repack_k(1, 0))
                    addf(10, lambda: repack_k(1, 1))
                    addf(10, lambda: q_dense(1, 0))
                    addf(11, lambda: q_dense(1, 1))
                    addf(12, lambda: repack_q(1))
                elif h in (1, 2, 3, 5):
                    fd = h + 1 if h != 5 else 5
                    if h == 2:
                        # h3 is a single straddle-free span at base row 32 of
                        # f2 (done during head 1) but still needs the
                        # partition shift to base 0
                        addf(1, lambda: repack_k(3, 0))
                        addf(2, lambda: repack_k(3, 1))
                        addf(3, lambda: repack_q(3))
                    for i, n in enumerate((0, 1, 2, 3)):
                        addf(2 * i + 1, lambda n=n, fd=fd: k_dense(fd, n))
                    addf(9, lambda fd=fd: q_dense(fd, 0))
                    addf(10, lambda fd=fd: q_dense(fd, 1))
                    # repack 2-span heads once their f_hi dense tiles exist
                    for hh in [hh for hh in range(H) if hh not in SINGLE_SPAN
                               and _f_hi(hh) == fd]:
                        addf(5, lambda hh=hh: repack_k(hh, 0))
                        addf(9, lambda hh=hh: repack_k(hh, 1))
                        addf(11, lambda hh=hh: repack_q(hh))
                if h in (1, 2, 3, 4):
                    for i in range(4):
                        addf(2 * i + 2, lambda tt=4 * (h - 1) + i: v_chunk(tt, 1))
                if h == 4:
                    for i, c in enumerate(range(CT)):
                        addf(2 * i + 5, lambda c=c: proj03(c, 0))
                if h == 6:
                    for i, c in enumerate(range(CT)):
                        addf(2 * i + 1, lambda c=c: proj03(c, 1))
                if h == 7:
                    # heads 4-6 qc1 projection during head-7 qc0 attention
                    # (8 groups); qc0 proj is appended inside attn_head7.
                    for i, c in enumerate(range(CT)):
                        addf(min(2 + i, 7), lambda c=c: proj46(c, 1))
                return f

            # norm(h) feeds only proj03/proj46/proj7 (heads 4+), so its
            # DVE ops are deferred ~2 heads so the gpsimd broadcast (and its
            # ucode lib load) never block the Vector FIFO.
            norm_slot = {0: (2, 8), 1: (3, 8), 2: (4, 1), 3: (4, 3),
                         4: (5, 8), 5: (6, 8), 6: (7, 1)}
            norms = {}
            fs = {h: mk_fillers(h) for h in range(H)}
            for h in range(H - 1):
                for hn, (ht, slot) in norm_slot.items():
                    if ht == h and hn in norms:
                        fs[h].setdefault(slot, []).insert(0, norms.pop(hn))
                norms[h] = attn_head(h, fs[h])
            f7 = fs[7]
            for hn, (ht, slot) in norm_slot.items():
                if ht == 7 and hn in norms:
                    f7.setdefault(slot, []).insert(0, norms.pop(hn))
            assert not norms, norms
            attn_head7(f7)

            # ---- tail: head-7 qc1 projection + out ----
            proj7(4, 0)
            proj7(5, 0)
            for ct in range(CT):
                proj7(ct, 1)

    nc.compile()
    return nc


def _get_nc():
    if "nc" not in _CACHE:
        _CACHE["nc"] = _build()
    return _CACHE["nc"]


def _prep_shards(x, w_qkv, w_proj, b_proj):
    bf16 = ml_dtypes.bfloat16
    x = np.asarray(x, dtype=np.float32)
    w_qkv = np.asarray(w_qkv, dtype=np.float32)
    w_proj = np.asarray(w_proj, dtype=np.float32)
    b_proj = np.asarray(b_proj, dtype=np.float32)

    def fmajor(w):  # [768c, 768f] -> [128, FT, CT, 128]: (p,f,ct,j) = w[ct*128+p, f*128+j]
        a = w.reshape(CT, 128, FT, 128)
        return np.ascontiguousarray(a.transpose(1, 2, 0, 3)).astype(bf16)

    def pmajor(w):  # [768c, F] -> [128, CT, F]
        return np.ascontiguousarray(
            w.reshape(CT, 128, w.shape[1]).transpose(1, 0, 2)
        ).astype(bf16)

    wq_b = fmajor(w_qkv[0:DIM].T * SCALE)
    wk_b = fmajor(w_qkv[DIM:2 * DIM].T)
    wv_b = pmajor(w_qkv[2 * DIM:3 * DIM].T)
    wp_arr = np.zeros((DH + 1, H, DIM), np.float32)
    wp_arr[0:DH] = w_proj.T.reshape(H, DH, DIM).transpose(1, 0, 2)
    wp_arr[DH, 0, :] = b_proj
    wp_b = np.ascontiguousarray(wp_arr).astype(bf16)

    in_maps = []
    for c in range(NCORES):
        b, half = divmod(c, 2)
        xt = x[b].T  # [768, 2048]
        if half == 1:
            xt = np.concatenate([xt[:, NQ:], xt[:, :NQ]], axis=1)
        xq = pmajor(xt)  # [128, CT, 2048]
        im = {
            "wka": np.ascontiguousarray(wk_b[:, 0:3]),
            "wkb": np.ascontiguousarray(wk_b[:, 3:6]),
            "wqa": np.ascontiguousarray(wq_b[:, 0:3]),
            "wqb": np.ascontiguousarray(wq_b[:, 3:6]),
            "wva": np.ascontiguousarray(wv_b[:, :, 0:4 * DH]),
            "wvb": np.ascontiguousarray(wv_b[:, :, 4 * DH:8 * DH]),
            "wp": wp_b,
        }
        for i in range(4):
            im[f"xq{i}"] = np.ascontiguousarray(xq[:, :, i * 512:(i + 1) * 512])
        in_maps.append(im)
    return in_maps


def kernel(x, w_qkv, w_proj, b_proj):
    from concourse.bass_utils import run_bass_kernel_spmd

    nc = _get_nc()
    in_maps = _prep_shards(x, w_qkv, w_proj, b_proj)
    res = run_bass_kernel_spmd(nc, in_maps, core_ids=list(range(NCORES)))
    out = np.empty((B, N, DIM), np.float32)
    for c in range(NCORES):
        b, half = divmod(c, 2)
        yT = np.asarray(res.results[c]["out"], dtype=np.float32)  # [768, 1024]
        out[b, half * NQ:(half + 1) * NQ, :] = yT.T
    return out
